# revision 15
# baseline (speedup 1.0000x reference)
"""Trainium2 Bass kernel for nn_NDT2_Transformer (B=8,T=1024,D=1024,H=16,depth=6,FF=4096).

Sharding: pure data-parallel — one batch element per NeuronCore (8 cores), no
collectives.  Per core the full 6-layer post-norm transformer encoder runs on a
S=1032-token sequence (1024 tokens + 8 ctx tokens).

On-chip layout: activations kept TRANSPOSED (features on partitions):
  xT  [128, 8, 1032] fp32   residual stream (d = chunk*128 + partition)
  xh  [128, 8, 1032] fp16   matmul-feed copy of xT (also reused for x^2 in LN)
Weights are host-pre-transposed to [in, out] so every linear is a
weight-stationary matmul producing transposed outputs with per-partition bias.

Attention uses the sortedness of `times`: the causal mask is a monotone
staircase, so (key-tile, query-tile) pairs are classified at trace time
(union over all 8 cores) as visible / staircase / skipped.  Scores are built
transposed (keys on partitions), softmax normalization is deferred via a
ones-column appended to V (sumexp rides along in the same matmul), applied as
a reciprocal partition-broadcast multiply when writing the per-head output.
"""
import os
import sys

sys.path.insert(0, "/opt/trn_rl_repo")

from contextlib import ExitStack

import numpy as np

import concourse.bass as bass
import concourse.mybir as mybir
import concourse.tile as tile
from concourse import bacc
from concourse.bass import ds, ts
from concourse.bass_utils import run_bass_kernel_spmd
from concourse.masks import make_identity

F32 = mybir.dt.float32
F16 = mybir.dt.float16
I32 = mybir.dt.int32

B, T, NCTX, D, H, DEPTH, FF = 8, 1024, 8, 1024, 16, 6, 4096
S = T + NCTX          # 1032
DH = D // H           # 64
NDC = D // 128        # 8 d-chunks
NFFC = FF // 128      # 32 ff-chunks
SCALE = 1.0 / float(np.sqrt(DH))
EPS = 1e-5
STS = [(0, 512), (512, 512), (1024, 8)]   # column tiles over S
NKT = T // 128        # 8 token key tiles; tile index NKT = ctx keys
Exp = mybir.ActivationFunctionType.Exp
Gelu = mybir.ActivationFunctionType.Gelu
Ident = mybir.ActivationFunctionType.Identity
Square = mybir.ActivationFunctionType.Square
MUL = mybir.AluOpType.mult
ADD = mybir.AluOpType.add
SUB = mybir.AluOpType.subtract
UNROLL_LAYERS = os.environ.get("KUNROLL", "1") == "1"
TWEAK = os.environ.get("KTWEAK", "1") == "1"
PAIR = os.environ.get("KPAIR", "1") == "1"
REPEAT = int(os.environ.get("KREPEAT", "1"))   # timing-only: loop layers N x
V2 = os.environ.get("KV2", "1") == "1"
V3 = os.environ.get("KV3", "0") == "1"


def _plan(times_all):
    """Trace-time (union over cores) staircase classification.

    qmin[b, k] = first query q with times[b, q] >= times[b, k];
    key k visible to query q iff q >= qmin[b, k]."""
    qmin = np.stack([np.searchsorted(t, t, side="left") for t in times_all])
    amin = [int(qmin[:, kt * 128].min()) for kt in range(NKT)]
    bmax = [int(qmin[:, kt * 128 + 127].max()) for kt in range(NKT)]
    token_plan = {}
    stair_kts = {}
    for qt in (0, 1):
        q0, w = STS[qt]
        q1 = q0 + w
        plan = []
        for kt in range(NKT):
            if amin[kt] >= q1:
                continue                      # fully masked everywhere
            if bmax[kt] <= q0:
                plan.append((kt, q0, 0, 0))   # fully visible
            else:
                c0 = max(amin[kt], q0)
                plan.append((kt, c0, c0, min(bmax[kt], q1)))
                stair_kts[kt] = (amin[kt], bmax[kt])
        token_plan[qt] = plan
    return qmin, token_plan, stair_kts


def _build(token_plan, stair_kts):
    nc = bacc.Bacc("TRN2", target_bir_lowering=False, debug=False, num_devices=B)
    d_src = nc.dram_tensor("src", [T, D], F32, kind="ExternalInput").ap()
    d_ctx = nc.dram_tensor("ctx", [NCTX, D], F32, kind="ExternalInput").ap()
    d_times = nc.dram_tensor("times_f", [1, T], F16, kind="ExternalInput").ap()
    d_spaces = nc.dram_tensor("spaces_f", [1, T], F16, kind="ExternalInput").ap()
    d_qmin = nc.dram_tensor("qmin_f", [128, NKT], F32, kind="ExternalInput").ap()
    d_temb = nc.dram_tensor("temb", [512, D], F16, kind="ExternalInput").ap()
    d_semb = nc.dram_tensor("semb", [64, D], F16, kind="ExternalInput").ap()
    d_qkvw = nc.dram_tensor("qkvw", [DEPTH * D, 3 * D], F16, kind="ExternalInput").ap()
    d_qkvb_qk = nc.dram_tensor("qkvb_qk", [DEPTH * 128, 16], F32, kind="ExternalInput").ap()
    d_brows = nc.dram_tensor("brows", [DEPTH, 3 * D], F16, kind="ExternalInput").ap()
    d_outw = nc.dram_tensor("outw", [DEPTH * D, D], F16, kind="ExternalInput").ap()
    d_l1w = nc.dram_tensor("l1w", [DEPTH * D, FF], F16, kind="ExternalInput").ap()
    d_l1b = nc.dram_tensor("l1b", [DEPTH * 128, NFFC], F32, kind="ExternalInput").ap()
    d_l2w = nc.dram_tensor("l2w", [DEPTH * FF, D], F16, kind="ExternalInput").ap()
    d_ln = nc.dram_tensor("lnp", [DEPTH * 128, 4 * NDC], F32, kind="ExternalInput").ap()
    d_out = nc.dram_tensor("out", [T, D], F32, kind="ExternalOutput").ap()

    with tile.TileContext(nc) as tc, ExitStack() as ctx:
        pers = ctx.enter_context(tc.tile_pool(name="pers", bufs=1))
        wpool = ctx.enter_context(tc.tile_pool(name="wpool", bufs=4 if TWEAK else 3))
        wvp = ctx.enter_context(tc.tile_pool(name="wvp", bufs=2 if TWEAK else 1))
        tmp = ctx.enter_context(tc.tile_pool(name="tmp", bufs=3))
        expp = ctx.enter_context(tc.tile_pool(name="expp", bufs=4 if V3 else 3))
        rowp = ctx.enter_context(tc.tile_pool(name="rowp", bufs=4 if V3 else 2))
        misc = ctx.enter_context(tc.tile_pool(name="misc", bufs=2))
        browp = ctx.enter_context(tc.tile_pool(name="browp", bufs=1))
        big = ctx.enter_context(tc.tile_pool(name="big", bufs=1))
        ps_acc = ctx.enter_context(tc.tile_pool(name="ps_acc", bufs=3, space="PSUM"))
        ps_att = ctx.enter_context(tc.tile_pool(name="ps_att", bufs=3, space="PSUM"))
        ps_o = ctx.enter_context(tc.tile_pool(name="ps_o", bufs=2, space="PSUM"))

        xT = pers.tile([128, NDC, S], F32, tag="xT")
        xh = pers.tile([128, NDC, S], F16, tag="xh")
        OT = pers.tile([128, NDC, S], F16, tag="OT")
        Vag = pers.tile([128, NKT + 1, H, DH + 1], F16, tag="Vag")
        ident = pers.tile([128, 128], F32, tag="ident")
        ones128 = pers.tile([128, 128], F16, tag="ones128")
        onesrow = pers.tile([1, 512], F16, tag="onesrow")
        qminT = pers.tile([128, NKT], F32, tag="qminT")
        masks = {}
        for kt, (a, b) in stair_kts.items():
            masks[kt] = pers.tile([128, b - a], F16, tag=f"mask{kt}", name=f"mask{kt}")

        make_identity(nc, ident[:])
        nc.gpsimd.memset(ones128[:], 1.0)
        nc.gpsimd.memset(onesrow[:], 1.0)
        nc.gpsimd.memset(Vag[:, :, :, DH : DH + 1], 1.0)
        nc.sync.dma_start(qminT[:], d_qmin)

        def acc_pool(i, name):
            """Alternate accumulation groups between the two 3-bank pools so
            consecutive tiles' PSUM groups overlap (ps_att is idle during the
            projection/FFN phases)."""
            if V2 and (i % 2 == 1):
                return ps_att.tile([128, 512], F32, tag="att", name=name)
            return ps_acc.tile([128, 512], F32, tag="acc", name=name)

        # ================= setup: masks + embeddings =================
        with tc.tile_pool(name="setup", bufs=1) as sp:
            io32 = sp.tile([128, T + NCTX], I32, tag="io32")
            iota = sp.tile([128, T + NCTX], F16, tag="iota")
            nc.gpsimd.iota(io32[:], pattern=[[1, S]], base=0, channel_multiplier=0)
            nc.vector.tensor_copy(iota[:], io32[:])
            for kt, (a, b) in stair_kts.items():
                nc.vector.tensor_scalar(
                    out=masks[kt][:], in0=iota[:, a:b],
                    scalar1=qminT[:, kt : kt + 1], scalar2=None,
                    op0=mybir.AluOpType.is_ge)
            vc32 = sp.tile([128, 4], I32, tag="vc32")
            vcol = sp.tile([128, 4], F32, tag="vcol")
            nc.gpsimd.iota(vc32[:], pattern=[[128, 4]], base=0, channel_multiplier=1)
            nc.vector.tensor_copy(vcol[:], vc32[:])
            trow = sp.tile([1, T], F16, tag="trow")
            srow = sp.tile([1, T], F16, tag="srow")
            nc.sync.dma_start(trow[:], d_times)
            nc.sync.dma_start(srow[:], d_spaces)

            for st in range(2):
                c0 = st * 512
                cs = slice(c0, c0 + 512)
                # broadcast times/spaces to all partitions (K=1 ones matmul)
                tbc = sp.tile([128, 512], F16, tag="tbc")
                sbc = sp.tile([128, 512], F16, tag="sbc")
                pt = ps_att.tile([128, 512], F32, tag="att")
                nc.tensor.matmul(pt[:], ones128[0:1, :], trow[0:1, cs],
                                 start=True, stop=True)
                nc.vector.tensor_copy(tbc[:], pt[:])
                pt2 = ps_att.tile([128, 512], F32, tag="att")
                nc.tensor.matmul(pt2[:], ones128[0:1, :], srow[0:1, cs],
                                 start=True, stop=True)
                nc.vector.tensor_copy(sbc[:], pt2[:])
                # onehots (exact in fp16: small integers)
                oht = sp.tile([128, 4, 512], F16, tag="oht")
                ohs = sp.tile([128, 512], F16, tag="ohs")
                for c in range(4):
                    nc.vector.tensor_scalar(
                        out=oht[:, c, :], in0=tbc[:], scalar1=vcol[:, c : c + 1],
                        scalar2=None, op0=mybir.AluOpType.is_equal)
                nc.vector.tensor_scalar(
                    out=ohs[:], in0=sbc[:], scalar1=vcol[:, 0:1],
                    scalar2=None, op0=mybir.AluOpType.is_equal)
                # x0 = (src + temb[times] + semb[spaces]).T for this 512-col block
                for dc in range(NDC):
                    dcs = slice(dc * 128, dc * 128 + 128)
                    tw = sp.tile([128, 4, 128], F16, tag="tw")
                    nc.sync.dma_start(
                        tw[:], d_temb[:, dcs].rearrange("(c p) m -> p c m", p=128))
                    sw = sp.tile([64, 128], F16, tag="sw")
                    nc.sync.dma_start(sw[:], d_semb[:, dcs])
                    px = ps_acc.tile([128, 512], F32, tag="acc")
                    for j in range(4):
                        stile = sp.tile([128, 128], F32, tag="srcst")
                        nc.sync.dma_start(
                            stile[:], d_src[c0 + j * 128 : c0 + j * 128 + 128, dcs])
                        nc.tensor.matmul(px[:, j * 128 : j * 128 + 128], stile[:],
                                         ident[:], is_transpose=True,
                                         start=(j == 0), stop=False)
                    for c in range(4):
                        nc.tensor.matmul(px[:], tw[:, c, :], oht[:, c, :],
                                         start=False, stop=False)
                    nc.tensor.matmul(px[:], sw[:], ohs[0:64, :],
                                     start=False, stop=True)
                    nc.vector.tensor_copy(xT[:, dc, cs], px[:])
                    nc.scalar.copy(xh[:, dc, cs], px[:])
            # ctx tokens -> cols 1024:1032
            for dc in range(NDC):
                dcs = slice(dc * 128, dc * 128 + 128)
                ct = sp.tile([8, 128], F32, tag="ct")
                nc.sync.dma_start(ct[:], d_ctx[:, dcs])
                pc = ps_acc.tile([128, 512], F32, tag="acc")
                nc.tensor.matmul(pc[:, 0:8], ct[:], ident[0:8, 0:8],
                                 is_transpose=True, start=True, stop=True)
                nc.vector.tensor_copy(xT[:, dc, T:S], pc[:, 0:8])
                nc.scalar.copy(xh[:, dc, T:S], pc[:, 0:8])

        # ================= transformer layers =================
        def layer_norm(g_ap, b_ap):
            for dc in range(NDC):
                nc.scalar.copy(xh[:, dc, :], xT[:, dc, :])
            for c0, w in STS:
                cs = slice(c0, c0 + w)
                psum = ps_acc.tile([128, 512], F32, tag="acc")
                for dc in range(NDC):
                    nc.tensor.matmul(psum[:, :w], ones128[:], xh[:, dc, cs],
                                     start=(dc == 0), stop=(dc == NDC - 1))
                psq = (ps_att if TWEAK else ps_acc).tile(
                    [128, 512], F32, tag="att" if TWEAK else "acc", name="psq")
                if V2:
                    for dc in range(NDC):
                        sq_t = tmp.tile([128, 512], F16, tag="sqt", name="sqt")
                        nc.scalar.activation(sq_t[:, :w], xT[:, dc, cs], Square)
                        nc.tensor.matmul(psq[:, :w], ones128[:], sq_t[:, :w],
                                         start=(dc == 0), stop=(dc == NDC - 1))
                else:
                    for dc in range(NDC):
                        nc.scalar.activation(xh[:, dc, cs], xT[:, dc, cs], Square)
                    for dc in range(NDC):
                        nc.tensor.matmul(psq[:, :w], ones128[:], xh[:, dc, cs],
                                         start=(dc == 0), stop=(dc == NDC - 1))
                # mS <- mean (SBUF), psq <- rstd (in place, replicated)
                mS = tmp.tile([128, 512], F32, tag="lnm")
                nc.vector.tensor_scalar(out=mS[:, :w], in0=psum[:, :w],
                                        scalar1=1.0 / D, scalar2=None, op0=MUL)
                m2 = tmp.tile([128, 512], F32, tag="lnt")
                nc.vector.tensor_tensor(out=m2[:, :w], in0=mS[:, :w],
                                        in1=mS[:, :w], op=MUL)
                nc.vector.tensor_scalar(out=psq[:, :w], in0=psq[:, :w],
                                        scalar1=1.0 / D, scalar2=EPS,
                                        op0=MUL, op1=ADD)
                nc.vector.tensor_tensor(out=psq[:, :w], in0=psq[:, :w],
                                        in1=m2[:, :w], op=SUB)
                nc.vector.reciprocal(psq[:, :w], psq[:, :w])
                nc.scalar.sqrt(psq[:, :w], psq[:, :w])
                for dc in range(NDC):
                    t = tmp.tile([128, 512], F32, tag="lnt")
                    nc.vector.tensor_tensor(out=t[:, :w], in0=xT[:, dc, cs],
                                            in1=mS[:, :w], op=SUB)
                    nc.vector.tensor_tensor(out=t[:, :w], in0=t[:, :w],
                                            in1=psq[:, :w], op=MUL)
                    nc.scalar.activation(xT[:, dc, cs], t[:, :w], Ident,
                                         bias=b_ap[:, dc : dc + 1],
                                         scale=g_ap[:, dc : dc + 1])
                    nc.scalar.copy(xh[:, dc, cs], xT[:, dc, cs])

        def build_layer(L):
            qkvb = misc.tile([128, 16], F32, tag="qkvb")
            nc.sync.dma_start(qkvb[:], d_qkvb_qk[ds(L * 128, 128), :])
            l1b = misc.tile([128, NFFC], F32, tag="l1b")
            nc.sync.dma_start(l1b[:], d_l1b[ds(L * 128, 128), :])
            lnp = misc.tile([128, 4 * NDC], F32, tag="lnp")
            nc.sync.dma_start(lnp[:], d_ln[ds(L * 128, 128), :])
            ln1s, ln1b = lnp[:, 0:NDC], lnp[:, NDC : 2 * NDC]
            ln2s, ln2b = lnp[:, 2 * NDC : 3 * NDC], lnp[:, 3 * NDC : 4 * NDC]
            brow = browp.tile([1, 3 * D], F16, tag="brows")
            nc.sync.dma_start(brow[:], d_brows[ds(L, 1), :])
            bias_v = brow[:, 0:D]
            bias_o = brow[:, D : 2 * D]
            bias_2 = brow[:, 2 * D : 3 * D]

            qkT = big.tile([128, 16, S], F16, tag="big")
            # ---- Q/K projections (transposed out), q/k interleaved per head-pair
            for m in [mm for hp in range(8) for mm in (hp, hp + 8)]:
                wsl = wpool.tile([128, NDC, 128], F16, tag="w")
                nc.sync.dma_start(
                    wsl[:], d_qkvw[ds(L * D, D), ts(m, 128)]
                    .rearrange("(ko p) m -> p ko m", p=128))
                for c0, w in STS:
                    cs = slice(c0, c0 + w)
                    pq = acc_pool(m, "pq")
                    for dc in range(NDC):
                        nc.tensor.matmul(pq[:, :w], wsl[:, dc, :], xh[:, dc, cs],
                                         start=(dc == 0), stop=(dc == NDC - 1))
                    nc.scalar.activation(qkT[:, m, cs], pq[:, :w], Ident,
                                         bias=qkvb[:, m : m + 1])
            # ---- V (normal layout, activation-stationary) + ones column
            for v in range(2):
                wv = wvp.tile([128, NDC, 512], F16, tag="wv")
                nc.sync.dma_start(
                    wv[:], d_qkvw[ds(L * D, D),
                                  2 * D + v * 512 : 2 * D + v * 512 + 512]
                    .rearrange("(ko p) m -> p ko m", p=128))
                for kt in range(NKT + 1):
                    m0 = kt * 128
                    mw = 128 if kt < NKT else 8
                    pv = acc_pool(kt, "pv")
                    for dc in range(NDC):
                        nc.tensor.matmul(pv[:mw, :], xh[:, dc, m0 : m0 + mw],
                                         wv[:, dc, :],
                                         start=(dc == 0), stop=False)
                    nc.tensor.matmul(pv[:mw, :], onesrow[0:1, 0:mw],
                                     bias_v[:, v * 512 : v * 512 + 512],
                                     start=False, stop=True)
                    nc.vector.tensor_copy(
                        Vag[:mw, kt, v * 8 : v * 8 + 8, 0:DH],
                        pv[:mw, :].rearrange("p (h e) -> p h e", e=DH))

            # ---- attention
            def attn_head_qt(hp, par, qt):
                """One (head, query-tile): scores -> exp (+mask) -> A@V -> OT."""
                h = 2 * hp + par
                b0 = 64 * par
                kv = qkT[b0 : b0 + 64, 8 + hp, :]
                qv = qkT[b0 : b0 + 64, hp, :]
                q0, qw = STS[qt]
                q1 = q0 + qw
                plan = token_plan[qt] if qt < 2 else []
                pso = ps_o.tile([65, 512], F32, tag="o", name=f"pso{par}")
                # ctx keys first (always visible; opens the psum group)
                pss = ps_att.tile([128, 512], F32, tag="att", name=f"pss{par}")
                nc.tensor.matmul(pss[0:8, :qw], kv[:, T:S], qv[:, q0:q1],
                                 start=True, stop=True)
                eT = expp.tile([128, 512], F16, tag="e", name=f"eT{par}")
                nc.scalar.activation(eT[0:8, :qw], pss[0:8, :qw], Exp,
                                     scale=SCALE)
                nc.tensor.matmul(pso[:, :qw], Vag[0:8, NKT, h, :],
                                 eT[0:8, :qw], start=True,
                                 stop=(len(plan) == 0))
                for i, (kt, c0, mlo, mhi) in enumerate(plan):
                    w = q1 - c0
                    ps2 = ps_att.tile([128, 512], F32, tag="att",
                                      name=f"ps2{par}")
                    nc.tensor.matmul(ps2[:, :w], kv[:, ts(kt, 128)],
                                     qv[:, c0:q1], start=True, stop=True)
                    e2 = expp.tile([128, 512], F16, tag="e", name=f"e2{par}")
                    nc.scalar.activation(e2[:, :w], ps2[:, :w], Exp,
                                         scale=SCALE)
                    if mhi > mlo:
                        a_kt = stair_kts[kt][0]
                        nc.vector.tensor_tensor(
                            out=e2[:, mlo - c0 : mhi - c0],
                            in0=e2[:, mlo - c0 : mhi - c0],
                            in1=masks[kt][:, mlo - a_kt : mhi - a_kt],
                            op=MUL)
                    nc.tensor.matmul(pso[:, c0 - q0 : qw],
                                     Vag[:, kt, h, :], e2[:, :w],
                                     start=False, stop=(i == len(plan) - 1))
                # normalize by sumexp (psum row 64) and write OT
                rr = rowp.tile([1, 512], F16, tag="rr", name=f"rr{par}")
                with nc.allow_low_precision(reason="softmax recip"):
                    nc.vector.reciprocal(rr[0:1, :qw], pso[64:65, :qw])
                rb = rowp.tile([64, 512], F16, tag="rb", name=f"rb{par}")
                nc.gpsimd.partition_broadcast(rb[:, :qw], rr[0:1, :qw])
                nc.vector.tensor_tensor(
                    out=OT[b0 : b0 + 64, hp, q0:q1],
                    in0=pso[0:64, :qw], in1=rb[:, :qw], op=MUL)

            def attn_pair_qt(hp, qt):
                """Both heads of a pair in lockstep: their K=64 score matmuls
                are issued back-to-back on disjoint PE row-groups (partitions
                0-63 vs 64-127) so the array runs them concurrently."""
                q0, qw = STS[qt]
                q1 = q0 + qw
                plan = token_plan[qt] if qt < 2 else []
                kvs = [qkT[64 * p : 64 * p + 64, 8 + hp, :] for p in (0, 1)]
                qvs = [qkT[64 * p : 64 * p + 64, hp, :] for p in (0, 1)]
                psos, eTs = [], []
                for p in (0, 1):
                    pso = ps_o.tile([65, 512], F32, tag="o", name=f"pso{p}")
                    psos.append(pso)
                psss = []
                for p in (0, 1):
                    pss = ps_att.tile([128, 512], F32, tag="att",
                                      name=f"pss{p}")
                    nc.tensor.matmul(pss[0:8, :qw], kvs[p][:, T:S],
                                     qvs[p][:, q0:q1], start=True, stop=True)
                    psss.append(pss)
                for p in (0, 1):
                    eT = expp.tile([128, 512], F16, tag="e", name=f"eT{p}")
                    nc.scalar.activation(eT[0:8, :qw], psss[p][0:8, :qw], Exp,
                                         scale=SCALE)
                    nc.tensor.matmul(psos[p][:, :qw],
                                     Vag[0:8, NKT, 2 * hp + p, :],
                                     eT[0:8, :qw], start=True,
                                     stop=(len(plan) == 0))
                for i, (kt, c0, mlo, mhi) in enumerate(plan):
                    w = q1 - c0
                    es = []
                    ps2s = []
                    for p in (0, 1):
                        ps2 = ps_att.tile([128, 512], F32, tag="att",
                                          name=f"ps2{p}")
                        nc.tensor.matmul(ps2[:, :w], kvs[p][:, ts(kt, 128)],
                                         qvs[p][:, c0:q1],
                                         start=True, stop=True)
                        ps2s.append(ps2)
                    for p in (0, 1):
                        e2 = expp.tile([128, 512], F16, tag="e",
                                       name=f"e2{p}")
                        nc.scalar.activation(e2[:, :w], ps2s[p][:, :w], Exp,
                                             scale=SCALE)
                        if mhi > mlo:
                            a_kt = stair_kts[kt][0]
                            nc.vector.tensor_tensor(
                                out=e2[:, mlo - c0 : mhi - c0],
                                in0=e2[:, mlo - c0 : mhi - c0],
                                in1=masks[kt][:, mlo - a_kt : mhi - a_kt],
                                op=MUL)
                        es.append(e2)
                    for p in (0, 1):
                        nc.tensor.matmul(psos[p][:, c0 - q0 : qw],
                                         Vag[:, kt, 2 * hp + p, :],
                                         es[p][:, :w], start=False,
                                         stop=(i == len(plan) - 1))
                for p in (0, 1):
                    rr = rowp.tile([1, 512], F16, tag="rr", name=f"rr{p}")
                    with nc.allow_low_precision(reason="softmax recip"):
                        nc.vector.reciprocal(rr[0:1, :qw],
                                             psos[p][64:65, :qw])
                    rb = rowp.tile([64, 512], F16, tag="rb", name=f"rb{p}")
                    nc.gpsimd.partition_broadcast(rb[:, :qw], rr[0:1, :qw])
                    nc.vector.tensor_tensor(
                        out=OT[64 * p : 64 * p + 64, hp, q0:q1],
                        in0=psos[p][0:64, :qw], in1=rb[:, :qw], op=MUL)

            if PAIR:
                for hp in range(8):
                    for qt in range(3):
                        attn_pair_qt(hp, qt)
            else:
                for hp in range(8):
                    for par in range(2):
                        for qt in range(3):
                            attn_head_qt(hp, par, qt)

            # ---- out projection (+ residual into xT)
            for m in range(NDC):
                wsl = wpool.tile([128, NDC, 128], F16, tag="w")
                nc.sync.dma_start(
                    wsl[:], d_outw[ds(L * D, D), ts(m, 128)]
                    .rearrange("(ko p) m -> p ko m", p=128))
                for c0, w in STS:
                    cs = slice(c0, c0 + w)
                    po = acc_pool(m, "po")
                    nc.tensor.matmul(po[:, :w], bias_o[:, ts(m, 128)],
                                     onesrow[0:1, :w], start=True, stop=False)
                    for c in range(NDC):
                        nc.tensor.matmul(po[:, :w], wsl[:, c, :], OT[:, c, cs],
                                         start=False, stop=(c == NDC - 1))
                    nc.vector.tensor_tensor(out=xT[:, m, cs], in0=xT[:, m, cs],
                                            in1=po[:, :w], op=ADD)
            layer_norm(ln1s, ln1b)

            # ---- FFN in two ff-halves (hT buffer is [128, 16, S])
            for half in range(2):
                hT = big.tile([128, NFFC // 2, S], F16, tag="big")
                for m in range(NFFC // 2):
                    mg = half * (NFFC // 2) + m
                    wsl = wpool.tile([128, NDC, 128], F16, tag="w")
                    nc.sync.dma_start(
                        wsl[:], d_l1w[ds(L * D, D), ts(mg, 128)]
                        .rearrange("(ko p) m -> p ko m", p=128))
                    for c0, w in STS:
                        cs = slice(c0, c0 + w)
                        pf = acc_pool(m, "pf")
                        for dc in range(NDC):
                            nc.tensor.matmul(pf[:, :w], wsl[:, dc, :],
                                             xh[:, dc, cs],
                                             start=(dc == 0), stop=(dc == NDC - 1))
                        nc.scalar.activation(hT[:, m, cs], pf[:, :w], Gelu,
                                             bias=l1b[:, mg : mg + 1])
                for m in range(NDC):
                    pf2s = [acc_pool(m, f"pf2_{sti}") for sti in range(3)]
                    if half == 0:
                        for sti, (c0, w) in enumerate(STS):
                            nc.tensor.matmul(pf2s[sti][:, :w],
                                             bias_2[:, ts(m, 128)],
                                             onesrow[0:1, :w],
                                             start=True, stop=False)
                    for kk in range(2):
                        wsl = wpool.tile([128, NDC, 128], F16, tag="w")
                        nc.sync.dma_start(
                            wsl[:], d_l2w[ds(L * FF + half * 2048 + kk * 1024, D),
                                          ts(m, 128)]
                            .rearrange("(ko p) m -> p ko m", p=128))
                        for sti, (c0, w) in enumerate(STS):
                            cs = slice(c0, c0 + w)
                            for c in range(NDC):
                                nc.tensor.matmul(
                                    pf2s[sti][:, :w], wsl[:, c, :],
                                    hT[:, kk * NDC + c, cs],
                                    start=(half == 1 and kk == 0 and c == 0),
                                    stop=(kk == 1 and c == NDC - 1))
                    for sti, (c0, w) in enumerate(STS):
                        cs = slice(c0, c0 + w)
                        nc.vector.tensor_tensor(out=xT[:, m, cs],
                                                in0=xT[:, m, cs],
                                                in1=pf2s[sti][:, :w], op=ADD)
            layer_norm(ln2s, ln2b)

        if UNROLL_LAYERS:
            for L in range(DEPTH):
                build_layer(L)
        else:
            with tc.For_i(0, DEPTH * REPEAT, 1) as L:
                build_layer(L % DEPTH if REPEAT > 1 else L)

        # ================= output: transpose token cols back ====
        for tt in range(T // 128):
            for half in range(2):
                po = ps_acc.tile([128, 512], F32, tag="acc")
                for j in range(4):
                    dc = half * 4 + j
                    nc.tensor.matmul(po[:, j * 128 : j * 128 + 128],
                                     xT[:, dc, ts(tt, 128)], ident[:],
                                     is_transpose=True,
                                     start=(j == 0), stop=(j == 3))
                ot = tmp.tile([128, 512], F32, tag="lnt")
                nc.vector.tensor_copy(ot[:], po[:])
                nc.sync.dma_start(
                    d_out[ts(tt, 128), half * 512 : half * 512 + 512], ot[:])

    nc.compile()
    return nc


_CACHE = {}


def _prep(src, ctx_emb, times, spaces, pad_mask, time_emb, space_emb,
          qkv_w, qkv_b, out_w, out_b, lin1_w, lin1_b, lin2_w, lin2_b,
          ln1_s, ln1_b, ln2_s, ln2_b):
    """Returns (nc, in_maps) — builds/caches the Bass program and the per-core
    input dicts (host work is layout/dtype prep only)."""
    src = np.asarray(src, np.float32)
    ctx_emb = np.asarray(ctx_emb, np.float32)
    times = np.asarray(times).astype(np.int64)
    spaces = np.asarray(spaces).astype(np.int64)
    f16 = lambda a: np.ascontiguousarray(np.asarray(a, np.float32).astype(np.float16))
    f32 = lambda a: np.ascontiguousarray(np.asarray(a, np.float32))

    key = times.tobytes()
    if key not in _CACHE:
        qmin, token_plan, stair_kts = _plan(times)
        _CACHE[key] = (_build(token_plan, stair_kts), qmin)
    nc, qmin = _CACHE[key]

    qkvw = f16(np.concatenate([np.asarray(qkv_w[i]).T for i in range(DEPTH)], 0))
    outw = f16(np.concatenate([np.asarray(out_w[i]).T for i in range(DEPTH)], 0))
    l1w = f16(np.concatenate([np.asarray(lin1_w[i]).T for i in range(DEPTH)], 0))
    l2w = f16(np.concatenate([np.asarray(lin2_w[i]).T for i in range(DEPTH)], 0))
    qkvb_qk = f32(np.concatenate(
        [np.asarray(qkv_b[i][: 2 * D], np.float32).reshape(16, 128).T
         for i in range(DEPTH)], 0))
    brows = f16(np.stack(
        [np.concatenate([np.asarray(qkv_b[i][2 * D :], np.float32),
                         np.asarray(out_b[i], np.float32),
                         np.asarray(lin2_b[i], np.float32)])
         for i in range(DEPTH)]))
    l1b = f32(np.concatenate(
        [np.asarray(lin1_b[i], np.float32).reshape(NFFC, 128).T
         for i in range(DEPTH)], 0))
    lnp = f32(np.concatenate(
        [np.concatenate(
            [np.asarray(p[i], np.float32).reshape(NDC, 128).T
             for p in (ln1_s, ln1_b, ln2_s, ln2_b)], 1)
         for i in range(DEPTH)], 0))
    temb = f16(time_emb)
    semb = f16(space_emb)

    in_maps = []
    for b in range(B):
        in_maps.append({
            "src": f32(src[b]),
            "ctx": f32(ctx_emb[b]),
            "times_f": f16(times[b].astype(np.float32)[None, :]),
            "spaces_f": f16(spaces[b].astype(np.float32)[None, :]),
            "qmin_f": f32(qmin[b].reshape(NKT, 128).T.astype(np.float32)),
            "temb": temb, "semb": semb,
            "qkvw": qkvw, "qkvb_qk": qkvb_qk, "brows": brows,
            "outw": outw, "l1w": l1w, "l1b": l1b, "l2w": l2w,
            "lnp": lnp,
        })
    return nc, in_maps


def kernel(_trace=False, **inputs):
    nc, in_maps = _prep(**inputs)
    res = run_bass_kernel_spmd(nc, in_maps, core_ids=list(range(B)),
                               trace=_trace)
    out = np.stack([r["out"] for r in res.results]).astype(np.float32)
    if _trace:
        kernel.last_exec_ns = res.exec_time_ns
        kernel.last_trace = res.instructions_and_trace
    return out
